# revision 1
# baseline (speedup 1.0000x reference)
"""Bass/Tile TRN2 kernel for nn_LongCatSelfAttention (8-core head-parallel).

Contract: kernel(**inputs) takes FULL unsharded inputs (as produced by the
problem's setup_inputs) and returns the FULL output [1, 3200, 3072] fp32.

Sharding: tensor-parallel over the 24 heads -> 3 heads per core. Each core
computes q/k/v projections for its heads, per-head RMSNorm + 3D RoPE,
split (cond/noise) attention, and a partial output through its slice of
Wo rows. The 8 partial outputs are summed on the host (no collectives).

All matmuls run as float32r (full PE rate, ~tf32 accuracy).
"""

import math

import numpy as np

import concourse.bacc as bacc
import concourse.mybir as mybir
import concourse.tile as tile
from concourse.bass_utils import run_bass_kernel_spmd

F32 = mybir.dt.float32
F32R = mybir.dt.float32r

# Problem constants (hardcoded per contract)
B = 1
T, H, W = 8, 20, 20
N = T * H * W  # 3200
DIM = 3072
NH = 24
HD = 128
D_T, D_H, D_W = 32, 48, 48
EPS = 1e-6
NCORES = 8
HPC = NH // NCORES  # heads per core = 3
HW_ = HPC * HD  # per-core head width = 384
KSUB = DIM // 128  # 24 contraction subtiles
P = 128

_PAIR_SWAP_MASK = [j ^ 1 for j in range(32)]


def _chunks(total, pref=512, min_sz=256):
    """Split `total` into chunks of `pref`, keeping every chunk >= min_sz
    (fp32r matmul runs at full rate only for free dim >= 256)."""
    if total <= pref:
        return [total]
    n, rem = divmod(total, pref)
    if rem == 0:
        return [pref] * n
    if rem >= min_sz:
        return [pref] * n + [rem]
    a = (pref + rem) // 2
    return [pref] * (n - 1) + [a, pref + rem - a]


def _ktiles(total):
    return [(k0, min(P, total - k0)) for k0 in range(0, total, P)]


def build_nc(nct):
    nc = bacc.Bacc()

    # ---- DRAM I/O ----
    xT = nc.dram_tensor("xT", [DIM, N], F32R, kind="ExternalInput")
    wq = nc.dram_tensor("wq", [DIM, HW_], F32R, kind="ExternalInput")
    wk = nc.dram_tensor("wk", [DIM, HW_], F32R, kind="ExternalInput")
    wv = nc.dram_tensor("wv", [DIM, HW_], F32R, kind="ExternalInput")
    wo = nc.dram_tensor("wo", [HW_, DIM], F32R, kind="ExternalInput")
    cw = nc.dram_tensor("cw", [P, N], F32, kind="ExternalInput")
    sw = nc.dram_tensor("sw", [P, N], F32, kind="ExternalInput")
    # per-head [128, HPC] vectors
    nwq = nc.dram_tensor("nwq", [P, 1], F32, kind="ExternalInput")  # q_norm_w
    nwk = nc.dram_tensor("nwk", [P, 1], F32, kind="ExternalInput")  # k_norm_w
    bqc = nc.dram_tensor("bqc", [P, HPC], F32, kind="ExternalInput")  # raw bq per head
    bkc = nc.dram_tensor("bkc", [P, HPC], F32, kind="ExternalInput")
    bvc = nc.dram_tensor("bvc", [P, HPC], F32, kind="ExternalInput")
    wbq = nc.dram_tensor("wbq", [P, HPC], F32, kind="ExternalInput")  # q_norm_w*bq
    wbk = nc.dram_tensor("wbk", [P, HPC], F32, kind="ExternalInput")
    ones_in = nc.dram_tensor("ones_in", [P, P], F32R, kind="ExternalInput")
    ident_in = nc.dram_tensor("ident_in", [P, P], F32, kind="ExternalInput")
    out = nc.dram_tensor("out", [N, DIM], F32, kind="ExternalOutput")

    # ---- DRAM staging (internal) ----
    qTd = nc.dram_tensor("qTd", [HPC, P, N], F32R, kind="Internal")
    kTd = nc.dram_tensor("kTd", [HPC, P, N], F32R, kind="Internal")
    vTd = nc.dram_tensor("vTd", [HPC, P, N], F32, kind="Internal")
    ctd = nc.dram_tensor("ctd", [HPC, P, N], F32R, kind="Internal")

    tchunks = _chunks(N)  # [512x5, 320, 320]
    segs = []  # (q0, qlen, klen) attention segments
    if nct > 0:
        segs.append((0, nct, nct))
    if nct < N:
        segs.append((nct, N - nct, N))

    with tile.TileContext(nc) as tc:
        with tc.tile_pool(name="const", bufs=1) as cpool:
            ones_sb = cpool.tile([P, P], F32R)
            ident_sb = cpool.tile([P, P], F32)
            nwq_sb = cpool.tile([P, 1], F32)
            nwk_sb = cpool.tile([P, 1], F32)
            bqc_sb = cpool.tile([P, HPC], F32)
            bkc_sb = cpool.tile([P, HPC], F32)
            bvc_sb = cpool.tile([P, HPC], F32)
            wbq_sb = cpool.tile([P, HPC], F32)
            wbk_sb = cpool.tile([P, HPC], F32)
            nc.sync.dma_start(ones_sb[:], ones_in[:])
            nc.sync.dma_start(ident_sb[:], ident_in[:])
            nc.sync.dma_start(nwq_sb[:], nwq[:])
            nc.sync.dma_start(nwk_sb[:], nwk[:])
            nc.sync.dma_start(bqc_sb[:], bqc[:])
            nc.sync.dma_start(bkc_sb[:], bkc[:])
            nc.sync.dma_start(bvc_sb[:], bvc[:])
            nc.sync.dma_start(wbq_sb[:], wbq[:])
            nc.sync.dma_start(wbk_sb[:], wbk[:])
            epsq_sb = cpool.tile([P, 1], F32)
            epsk_sb = cpool.tile([P, 1], F32)
            nc.vector.memset(epsq_sb[:], float(HD) * EPS)
            nc.vector.memset(epsk_sb[:], EPS)

            # ================= PHASE 1: QKV + norm + rope =================
            with tc.tile_pool(name="p1w", bufs=1) as wpool, \
                 tc.tile_pool(name="p1x", bufs=28) as xpool, \
                 tc.tile_pool(name="p1t", bufs=2) as tpool, \
                 tc.tile_pool(name="p1wk", bufs=2) as kpool, \
                 tc.tile_pool(name="p1ps", bufs=2, space="PSUM") as pspool, \
                 tc.tile_pool(name="p1ps2", bufs=2, space="PSUM") as pspool2:
                wq_sb = wpool.tile([P, KSUB, HW_], F32R, tag="wq")
                wk_sb = wpool.tile([P, KSUB, HW_], F32R, tag="wk")
                wv_sb = wpool.tile([P, KSUB, HW_], F32R, tag="wv")
                nc.sync.dma_start(wq_sb[:], wq.rearrange("(ko p) m -> p ko m", p=P))
                nc.sync.dma_start(wk_sb[:], wk.rearrange("(ko p) m -> p ko m", p=P))
                nc.sync.dma_start(wv_sb[:], wv.rearrange("(ko p) m -> p ko m", p=P))

                t0 = 0
                for tcw in tchunks:
                    xs = []
                    for k in range(KSUB):
                        xt = xpool.tile([P, 512], F32R, tag="x")
                        nc.sync.dma_start(
                            xt[:, :tcw], xT[k * P : (k + 1) * P, t0 : t0 + tcw]
                        )
                        xs.append(xt)
                    cw_t = tpool.tile([P, 512], F32, tag="cw")
                    sw_t = tpool.tile([P, 512], F32, tag="sw")
                    nc.sync.dma_start(cw_t[:, :tcw], cw[:, t0 : t0 + tcw])
                    nc.sync.dma_start(sw_t[:, :tcw], sw[:, t0 : t0 + tcw])

                    for h in range(HPC):
                        for proj, w_sb, stage in (
                            ("q", wq_sb, qTd),
                            ("k", wk_sb, kTd),
                            ("v", wv_sb, vTd),
                        ):
                            ps = pspool.tile([P, 512], F32, tag="qkv")
                            for k in range(KSUB):
                                nc.tensor.matmul(
                                    ps[:, :tcw],
                                    w_sb[:, k, h * HD : (h + 1) * HD],
                                    xs[k][:, :tcw],
                                    start=(k == 0),
                                    stop=(k == KSUB - 1),
                                )
                            if proj == "v":
                                vt = kpool.tile([P, 512], F32, tag="qbw")
                                nc.scalar.activation(
                                    vt[:, :tcw],
                                    ps[:, :tcw],
                                    mybir.ActivationFunctionType.Identity,
                                    bias=bvc_sb[:, h : h + 1],
                                )
                                nc.sync.dma_start(
                                    stage[h, :, t0 : t0 + tcw], vt[:, :tcw]
                                )
                                continue
                            if proj == "q":
                                nw_sb, wb_sb, b_sb = nwq_sb, wbq_sb, bqc_sb
                                sq_scale, sq_bias = 1.0, epsq_sb
                            else:
                                nw_sb, wb_sb, b_sb = nwk_sb, wbk_sb, bkc_sb
                                sq_scale, sq_bias = 1.0 / HD, epsk_sb
                            # qbw = w*(q+b) ; qsq = (q+b)^2
                            qbw = kpool.tile([P, 512], F32, tag="qbw")
                            nc.scalar.activation(
                                qbw[:, :tcw],
                                ps[:, :tcw],
                                mybir.ActivationFunctionType.Identity,
                                bias=wb_sb[:, h : h + 1],
                                scale=nw_sb[:],
                            )
                            qsq = kpool.tile([P, 512], F32R, tag="qsq")
                            nc.scalar.activation(
                                qsq[:, :tcw],
                                ps[:, :tcw],
                                mybir.ActivationFunctionType.Square,
                                bias=b_sb[:, h : h + 1],
                            )
                            # replicated sumsq over hd via ones matmul
                            ssq = pspool2.tile([P, 512], F32, tag="ssq")
                            nc.tensor.matmul(
                                ssq[:, :tcw],
                                ones_sb[:],
                                qsq[:, :tcw],
                                start=True,
                                stop=True,
                            )
                            rmst = kpool.tile([P, 512], F32, tag="rms")
                            nc.scalar.activation(
                                rmst[:, :tcw],
                                ssq[:, :tcw],
                                mybir.ActivationFunctionType.Sqrt,
                                bias=sq_bias[:],
                                scale=sq_scale,
                            )
                            rcp = kpool.tile([P, 512], F32, tag="rcp")
                            nc.vector.reciprocal(rcp[:, :tcw], rmst[:, :tcw])
                            # rope: (qbw*cw + shuffle(qbw)*sw) * rcp
                            qsw = kpool.tile([P, 512], F32, tag="qsw")
                            nc.vector.stream_shuffle(
                                qsw[:, :tcw], qbw[:, :tcw], _PAIR_SWAP_MASK
                            )
                            m1 = kpool.tile([P, 512], F32, tag="m1")
                            nc.vector.tensor_tensor(
                                m1[:, :tcw], qbw[:, :tcw], cw_t[:, :tcw],
                                mybir.AluOpType.mult,
                            )
                            m2 = kpool.tile([P, 512], F32, tag="m2")
                            nc.vector.tensor_tensor(
                                m2[:, :tcw], qsw[:, :tcw], sw_t[:, :tcw],
                                mybir.AluOpType.mult,
                            )
                            nc.vector.tensor_tensor(
                                m1[:, :tcw], m1[:, :tcw], m2[:, :tcw],
                                mybir.AluOpType.add,
                            )
                            qfin = kpool.tile([P, 512], F32R, tag="qfin")
                            nc.vector.tensor_tensor(
                                qfin[:, :tcw], m1[:, :tcw], rcp[:, :tcw],
                                mybir.AluOpType.mult,
                            )
                            nc.sync.dma_start(
                                stage[h, :, t0 : t0 + tcw], qfin[:, :tcw]
                            )
                    t0 += tcw

            # ================= PHASE 2: attention =================
            with tc.tile_pool(name="p2qkv", bufs=2) as qkvp, \
                 tc.tile_pool(name="p2w", bufs=4) as wkp, \
                 tc.tile_pool(name="p2ps", bufs=2, space="PSUM") as psS, \
                 tc.tile_pool(name="p2pc", bufs=2, space="PSUM") as psC, \
                 tc.tile_pool(name="p2pt", bufs=1, space="PSUM") as psT:
                for h in range(HPC):
                    kT_sb = qkvp.tile([P, N], F32R, tag="kT")
                    qT_sb = qkvp.tile([P, N], F32R, tag="qT")
                    vT_sb = qkvp.tile([P, N], F32, tag="vT")
                    vn_sb = qkvp.tile([P, KSUB + 1, P], F32R, tag="vn")
                    nc.sync.dma_start(kT_sb[:], kTd[h])
                    nc.sync.dma_start(qT_sb[:], qTd[h])
                    nc.sync.dma_start(vT_sb[:], vTd[h])
                    # transpose V.T -> V natural [tok, hd] per 128-tile
                    for i, (k0, ksz) in enumerate(_ktiles(N)):
                        pst = psT.tile([P, P], F32, tag="tp")
                        nc.tensor.transpose(
                            pst[:ksz, :], vT_sb[:, k0 : k0 + ksz], ident_sb[:]
                        )
                        nc.scalar.copy(vn_sb[:ksz, i, :], pst[:ksz, :])

                    for q0, qlen, klen in segs:
                        kts = _ktiles(klen)
                        qc0 = 0
                        for qcw in _chunks(qlen):
                            qs = q0 + qc0
                            ct_ps = psC.tile([P, 512], F32, tag="ct")
                            dn_ps = psC.tile([P, 512], F32, tag="dn")
                            for i, (k0, ksz) in enumerate(kts):
                                st = psS.tile([P, 512], F32, tag="st")
                                nc.tensor.matmul(
                                    st[:ksz, :qcw],
                                    kT_sb[:, k0 : k0 + ksz],
                                    qT_sb[:, qs : qs + qcw],
                                    start=True,
                                    stop=True,
                                )
                                pt = wkp.tile([P, 512], F32R, tag="pt")
                                nc.scalar.activation(
                                    pt[:ksz, :qcw],
                                    st[:ksz, :qcw],
                                    mybir.ActivationFunctionType.Exp,
                                )
                                nc.tensor.matmul(
                                    ct_ps[:, :qcw],
                                    vn_sb[:ksz, i, :],
                                    pt[:ksz, :qcw],
                                    start=(i == 0),
                                    stop=(i == len(kts) - 1),
                                )
                                nc.tensor.matmul(
                                    dn_ps[:, :qcw],
                                    ones_sb[:ksz, :],
                                    pt[:ksz, :qcw],
                                    start=(i == 0),
                                    stop=(i == len(kts) - 1),
                                )
                            rcp = wkp.tile([P, 512], F32, tag="rcp2")
                            nc.vector.reciprocal(rcp[:, :qcw], dn_ps[:, :qcw])
                            ctt = wkp.tile([P, 512], F32R, tag="ctt")
                            nc.vector.tensor_tensor(
                                ctt[:, :qcw], ct_ps[:, :qcw], rcp[:, :qcw],
                                mybir.AluOpType.mult,
                            )
                            nc.sync.dma_start(
                                ctd[h, :, qs : qs + qcw], ctt[:, :qcw]
                            )
                            qc0 += qcw

            # ================= PHASE 3: out projection =================
            with tc.tile_pool(name="p3wo", bufs=1) as wop, \
                 tc.tile_pool(name="p3ct", bufs=2) as ctp, \
                 tc.tile_pool(name="p3o", bufs=3) as outp, \
                 tc.tile_pool(name="p3ps", bufs=2, space="PSUM") as psO:
                wo_sb = wop.tile([P, HPC, DIM], F32R)
                nc.sync.dma_start(wo_sb[:], wo.rearrange("(h p) o -> p h o", p=P))
                ts0 = 0
                for tsw in _chunks(N, 512, 128):
                    ct3 = ctp.tile([P, HPC, 512], F32R, tag="ct3")
                    for h in range(HPC):
                        nc.sync.dma_start(
                            ct3[:, h, :tsw], ctd[h, :, ts0 : ts0 + tsw]
                        )
                    for tt in range(0, tsw, P):
                        ttw = min(P, tsw - tt)
                        for oc in range(DIM // 512):
                            po = psO.tile([P, 512], F32, tag="po")
                            for h in range(HPC):
                                nc.tensor.matmul(
                                    po[:ttw, :],
                                    ct3[:, h, tt : tt + ttw],
                                    wo_sb[:, h, oc * 512 : (oc + 1) * 512],
                                    start=(h == 0),
                                    stop=(h == HPC - 1),
                                )
                            ot = outp.tile([P, 512], F32, tag="ot")
                            nc.scalar.copy(ot[:ttw, :], po[:ttw, :])
                            nc.sync.dma_start(
                                out[ts0 + tt : ts0 + tt + ttw,
                                    oc * 512 : (oc + 1) * 512],
                                ot[:ttw, :],
                            )
                    ts0 += tsw

    nc.compile()
    return nc


def _rope_tables():
    def axis_freqs(d, n):
        inv = 1.0 / (10000.0 ** (np.arange(0, d, 2, dtype=np.float32) / d))
        return np.arange(n, dtype=np.float32)[:, None] * inv[None, :]

    ft = np.broadcast_to(
        axis_freqs(D_T, T)[:, None, None, :], (T, H, W, D_T // 2)
    )
    fh = np.broadcast_to(
        axis_freqs(D_H, H)[None, :, None, :], (T, H, W, D_H // 2)
    )
    fw = np.broadcast_to(
        axis_freqs(D_W, W)[None, None, :, :], (T, H, W, D_W // 2)
    )
    f = np.concatenate([ft, fh, fw], axis=-1).reshape(N, HD // 2)
    cos = np.cos(f).astype(np.float32)  # [N, 64]
    sin = np.sin(f).astype(np.float32)
    cwt = np.repeat(cos.T, 2, axis=0)  # [128, N]
    swt = np.empty((HD, N), np.float32)
    swt[0::2] = -sin.T
    swt[1::2] = sin.T
    return cwt, swt


_NC_CACHE = {}


def kernel(**inputs):
    x = np.asarray(inputs["x"], dtype=np.float32)
    Wq = np.asarray(inputs["Wq"], dtype=np.float32)
    bq = np.asarray(inputs["bq"], dtype=np.float32)
    Wk = np.asarray(inputs["Wk"], dtype=np.float32)
    bk = np.asarray(inputs["bk"], dtype=np.float32)
    Wv = np.asarray(inputs["Wv"], dtype=np.float32)
    bv = np.asarray(inputs["bv"], dtype=np.float32)
    Wo = np.asarray(inputs["Wo"], dtype=np.float32)
    bo = np.asarray(inputs["bo"], dtype=np.float32)
    qnw = np.asarray(inputs["q_norm_w"], dtype=np.float32)
    knw = np.asarray(inputs["k_norm_w"], dtype=np.float32)
    nct = int(inputs["num_cond_latents"]) * (N // T)

    if nct not in _NC_CACHE:
        _NC_CACHE[nct] = build_nc(nct)
    nc = _NC_CACHE[nct]

    xT = np.ascontiguousarray(x.reshape(N, DIM).T)
    cwt, swt = _rope_tables()
    ones = np.ones((P, P), np.float32)
    ident = np.eye(P, dtype=np.float32)

    def headcols(vec, c):
        return np.ascontiguousarray(
            vec[c * HW_ : (c + 1) * HW_].reshape(HPC, HD).T
        )

    in_maps = []
    for c in range(NCORES):
        sl = slice(c * HW_, (c + 1) * HW_)
        in_maps.append(
            {
                "xT": xT,
                "wq": np.ascontiguousarray(Wq[:, sl]),
                "wk": np.ascontiguousarray(Wk[:, sl]),
                "wv": np.ascontiguousarray(Wv[:, sl]),
                "wo": np.ascontiguousarray(Wo[sl, :]),
                "cw": cwt,
                "sw": swt,
                "nwq": qnw.reshape(P, 1),
                "nwk": knw.reshape(P, 1),
                "bqc": headcols(bq, c),
                "bkc": headcols(bk, c),
                "bvc": headcols(bv, c),
                "wbq": headcols(bq, c) * qnw.reshape(P, 1),
                "wbk": headcols(bk, c) * knw.reshape(P, 1),
                "ones_in": ones,
                "ident_in": ident,
            }
        )

    res = run_bass_kernel_spmd(nc, in_maps, core_ids=list(range(NCORES)))
    acc = np.zeros((N, DIM), np.float64)
    for c in range(NCORES):
        acc += res.results[c]["out"]
    result = (acc + bo).astype(np.float32).reshape(B, N, DIM)
    return result


if __name__ == "__main__":
    rng = np.random.default_rng(0)
    build_nc(800)
    print("build ok")



# revision 2
# speedup vs baseline: 81119.2257x; 81119.2257x over previous
"""Bass/Tile TRN2 kernel for nn_LongCatSelfAttention (8-core head-parallel, fused
+ software-pipelined emission).

Same algorithm as kernel_v2 (bf16 matmuls, SBUF-resident q/k/v, fused
attention + out-projection), plus deferred-emission scheduling to keep the
TensorEngine busy through ScalarE (exp / rmsnorm) latencies:
  - phase 1: the k-projection rms matmul + tail and the v-projection units
    are deferred one head so the PE never waits on ACT.
  - phase 2/3: out-projection units of query-chunk j-1 are interleaved into
    the exp-wait gaps of chunk j's attention; PSUM drains happen on VectorE
    (ACT is the attention bottleneck).
"""

import numpy as np

import concourse.bacc as bacc
import concourse.mybir as mybir
import concourse.tile as tile
from concourse.bass_utils import run_bass_kernel_spmd

F32 = mybir.dt.float32
F32R = mybir.dt.float32r
BF16 = mybir.dt.bfloat16

B = 1
T, H, W = 8, 20, 20
N = T * H * W  # 3200
DIM = 3072
NH = 24
HD = 128
D_T, D_H, D_W = 32, 48, 48
EPS = 1e-6
NCORES = 8
HPC = NH // NCORES
HW_ = HPC * HD  # 384
KSUB = DIM // 128  # 24
P = 128

_PAIR_SWAP_MASK = [j ^ 1 for j in range(32)]

P1_CHUNKS = [384] * 8 + [128]
P1_MAX = max(P1_CHUNKS)
assert sum(P1_CHUNKS) == N


def _ktiles(total):
    return [(k0, min(P, total - k0)) for k0 in range(0, total, P)]


def _qchunks(nct):
    out = []
    for seg0, seglen, klen in ((0, nct, nct), (nct, N - nct, N)):
        q0 = seg0
        while q0 < seg0 + seglen:
            qlen = min(512, seg0 + seglen - q0)
            out.append((q0, qlen, klen))
            q0 += qlen
    return out


def build_nc(nct, iters=1, phases=3):
    nc = bacc.Bacc()

    xT = nc.dram_tensor("xT", [DIM, N], BF16, kind="ExternalInput")
    wq = nc.dram_tensor("wq", [DIM, HW_], BF16, kind="ExternalInput")
    wk = nc.dram_tensor("wk", [DIM, HW_], BF16, kind="ExternalInput")
    wv = nc.dram_tensor("wv", [DIM, HW_], BF16, kind="ExternalInput")
    wo = nc.dram_tensor("wo", [HW_, DIM], BF16, kind="ExternalInput")
    cw = nc.dram_tensor("cw", [P, N], F32, kind="ExternalInput")
    sw = nc.dram_tensor("sw", [P, N], F32, kind="ExternalInput")
    nwq = nc.dram_tensor("nwq", [P, 1], F32, kind="ExternalInput")
    nwk = nc.dram_tensor("nwk", [P, 1], F32, kind="ExternalInput")
    bqc = nc.dram_tensor("bqc", [P, HPC], F32, kind="ExternalInput")
    bkc = nc.dram_tensor("bkc", [P, HPC], F32, kind="ExternalInput")
    wbq = nc.dram_tensor("wbq", [P, HPC], F32, kind="ExternalInput")
    wbk = nc.dram_tensor("wbk", [P, HPC], F32, kind="ExternalInput")
    bvr = nc.dram_tensor("bvr", [1, HW_], BF16, kind="ExternalInput")
    ones_bf_in = nc.dram_tensor("ones_bf", [P, P], BF16, kind="ExternalInput")
    ones_f_in = nc.dram_tensor("ones_f", [P, P], F32R, kind="ExternalInput")
    out = nc.dram_tensor("out", [N, DIM], F32, kind="ExternalOutput")

    qcs = _qchunks(nct)
    n_vt = N // P

    with tile.TileContext(nc) as tc:
        with tc.tile_pool(name="const", bufs=1) as cpool:
            ones_bf = cpool.tile([P, P], BF16)
            ones_f = cpool.tile([P, P], F32R)
            nwq_sb = cpool.tile([P, 1], F32)
            nwk_sb = cpool.tile([P, 1], F32)
            bqc_sb = cpool.tile([P, HPC], F32)
            bkc_sb = cpool.tile([P, HPC], F32)
            wbq_sb = cpool.tile([P, HPC], F32)
            wbk_sb = cpool.tile([P, HPC], F32)
            bvr_sb = cpool.tile([1, HW_], BF16)
            nc.sync.dma_start(ones_bf[:], ones_bf_in[:])
            nc.sync.dma_start(ones_f[:], ones_f_in[:])
            nc.sync.dma_start(nwq_sb[:], nwq[:])
            nc.sync.dma_start(nwk_sb[:], nwk[:])
            nc.sync.dma_start(bqc_sb[:], bqc[:])
            nc.sync.dma_start(bkc_sb[:], bkc[:])
            nc.sync.dma_start(wbq_sb[:], wbq[:])
            nc.sync.dma_start(wbk_sb[:], wbk[:])
            nc.sync.dma_start(bvr_sb[:], bvr[:])
            epsq_sb = cpool.tile([P, 1], F32)
            epsk_sb = cpool.tile([P, 1], F32)
            nc.vector.memset(epsq_sb[:], float(HD) * EPS)
            nc.vector.memset(epsk_sb[:], EPS)

            for it in range(iters):
                with tc.tile_pool(name=f"qkv{it}", bufs=1) as qkvpool:
                    qs = qkvpool.tile([P, HPC, N], BF16, tag="qs")
                    ks = qkvpool.tile([P, HPC, N], BF16, tag="ks")
                    vn = qkvpool.tile([P, n_vt, HW_], BF16, tag="vn")

                    # ============ PHASE 1: QKV + norm + rope ============
                    with tc.tile_pool(name=f"p1w{it}", bufs=1) as wpool, \
                         tc.tile_pool(name=f"p1x{it}", bufs=2) as xpool, \
                         tc.tile_pool(name=f"p1t{it}", bufs=2) as tpool, \
                         tc.tile_pool(name=f"p1k{it}", bufs=3) as kpool, \
                         tc.tile_pool(name=f"p1ps{it}", bufs=2, space="PSUM") as pspool, \
                         tc.tile_pool(name=f"p1ps2{it}", bufs=2, space="PSUM") as pspool2, \
                         tc.tile_pool(name=f"p1psv{it}", bufs=2, space="PSUM") as psvpool:
                        wq_sb = wpool.tile([P, KSUB, HW_], BF16, tag="wq")
                        wk_sb = wpool.tile([P, KSUB, HW_], BF16, tag="wk")
                        wv_sb = wpool.tile([P, KSUB, HW_], BF16, tag="wv")
                        wqr = wq.rearrange("(ko p) m -> p ko m", p=P)
                        wkr = wk.rearrange("(ko p) m -> p ko m", p=P)
                        wvr = wv.rearrange("(ko p) m -> p ko m", p=P)
                        for kg in range(0, KSUB, 4):
                            nc.sync.dma_start(wq_sb[:, kg : kg + 4, :], wqr[:, kg : kg + 4, :])
                            nc.sync.dma_start(wk_sb[:, kg : kg + 4, :], wkr[:, kg : kg + 4, :])
                            nc.sync.dma_start(wv_sb[:, kg : kg + 4, :], wvr[:, kg : kg + 4, :])

                        deferred = []

                        def drain():
                            while deferred:
                                deferred.pop(0)()

                        def norm_tail(ps, qsq, qbw, cw_t, sw_t, dst, h, t0, tcw,
                                      sq_scale, sq_bias):
                            # rms matmul + sqrt/recip + rope, emitted as a unit
                            ssq = pspool2.tile([P, P1_MAX], F32, tag="ssq")
                            nc.tensor.matmul(
                                ssq[:, :tcw], ones_f[:], qsq[:, :tcw],
                                start=True, stop=True,
                            )
                            rmst = kpool.tile([P, P1_MAX], F32, tag="rms")
                            nc.scalar.activation(
                                rmst[:, :tcw], ssq[:, :tcw],
                                mybir.ActivationFunctionType.Sqrt,
                                bias=sq_bias[:], scale=sq_scale,
                            )
                            rcp = kpool.tile([P, P1_MAX], F32, tag="rcp")
                            nc.vector.reciprocal(rcp[:, :tcw], rmst[:, :tcw])
                            qsw = kpool.tile([P, P1_MAX], F32, tag="qsw")
                            nc.vector.stream_shuffle(
                                qsw[:, :tcw], qbw[:, :tcw], _PAIR_SWAP_MASK
                            )
                            m1 = kpool.tile([P, P1_MAX], F32, tag="m1")
                            nc.vector.tensor_tensor(
                                m1[:, :tcw], qbw[:, :tcw], cw_t[:, :tcw],
                                mybir.AluOpType.mult,
                            )
                            m2 = kpool.tile([P, P1_MAX], F32, tag="m2")
                            nc.vector.tensor_tensor(
                                m2[:, :tcw], qsw[:, :tcw], sw_t[:, :tcw],
                                mybir.AluOpType.mult,
                            )
                            nc.vector.tensor_tensor(
                                m1[:, :tcw], m1[:, :tcw], m2[:, :tcw],
                                mybir.AluOpType.add,
                            )
                            nc.vector.tensor_tensor(
                                dst[:, h, t0 : t0 + tcw], m1[:, :tcw],
                                rcp[:, :tcw], mybir.AluOpType.mult,
                            )

                        def v_unit(xt, tt, ttw, ti):
                            psv = psvpool.tile([P, HW_], F32, tag="v")
                            for k in range(KSUB):
                                nc.tensor.matmul(
                                    psv[:ttw, :], xt[:, k, tt : tt + ttw],
                                    wv_sb[:, k, :], start=(k == 0), stop=False,
                                )
                            nc.tensor.matmul(
                                psv[:ttw, :], ones_bf[:1, :ttw], bvr_sb[:],
                                start=False, stop=True,
                            )
                            nc.scalar.copy(vn[:ttw, ti, :], psv[:ttw, :])

                        t0 = 0
                        for tcw in P1_CHUNKS:
                            xt = xpool.tile([P, KSUB, P1_MAX], BF16, tag="x")
                            xr = xT.rearrange("(ko p) n -> p ko n", p=P)
                            for kg in range(0, KSUB, 6):
                                nc.sync.dma_start(
                                    xt[:, kg : kg + 6, :tcw],
                                    xr[:, kg : kg + 6, t0 : t0 + tcw],
                                )
                            cw_t = tpool.tile([P, P1_MAX], F32, tag="cw")
                            sw_t = tpool.tile([P, P1_MAX], F32, tag="sw")
                            nc.sync.dma_start(cw_t[:, :tcw], cw[:, t0 : t0 + tcw])
                            nc.sync.dma_start(sw_t[:, :tcw], sw[:, t0 : t0 + tcw])

                            vtiles = [(tt, min(P, tcw - tt), (t0 + tt) // P)
                                      for tt in range(0, tcw, P)]
                            for h in range(HPC):
                                # qk projection matmuls back to back
                                ps_q = pspool.tile([P, P1_MAX], F32, tag="psq")
                                for k in range(KSUB):
                                    nc.tensor.matmul(
                                        ps_q[:, :tcw],
                                        wq_sb[:, k, h * HD : (h + 1) * HD],
                                        xt[:, k, :tcw],
                                        start=(k == 0), stop=(k == KSUB - 1),
                                    )
                                ps_k = pspool.tile([P, P1_MAX], F32, tag="psk")
                                for k in range(KSUB):
                                    nc.tensor.matmul(
                                        ps_k[:, :tcw],
                                        wk_sb[:, k, h * HD : (h + 1) * HD],
                                        xt[:, k, :tcw],
                                        start=(k == 0), stop=(k == KSUB - 1),
                                    )
                                # ACT ops for q then k (overlap the matmuls above)
                                qbw_q = kpool.tile([P, P1_MAX], F32, tag="qbw")
                                nc.scalar.activation(
                                    qbw_q[:, :tcw], ps_q[:, :tcw],
                                    mybir.ActivationFunctionType.Identity,
                                    bias=wbq_sb[:, h : h + 1], scale=nwq_sb[:],
                                )
                                qsq_q = kpool.tile([P, P1_MAX], F32R, tag="qsq")
                                nc.scalar.activation(
                                    qsq_q[:, :tcw], ps_q[:, :tcw],
                                    mybir.ActivationFunctionType.Square,
                                    bias=bqc_sb[:, h : h + 1],
                                )
                                qbw_k = kpool.tile([P, P1_MAX], F32, tag="qbw")
                                nc.scalar.activation(
                                    qbw_k[:, :tcw], ps_k[:, :tcw],
                                    mybir.ActivationFunctionType.Identity,
                                    bias=wbk_sb[:, h : h + 1], scale=nwk_sb[:],
                                )
                                qsq_k = kpool.tile([P, P1_MAX], F32R, tag="qsq")
                                nc.scalar.activation(
                                    qsq_k[:, :tcw], ps_k[:, :tcw],
                                    mybir.ActivationFunctionType.Square,
                                    bias=bkc_sb[:, h : h + 1],
                                )
                                # q tail now (ACT had the k-matmul window)
                                norm_tail(ps_q, qsq_q, qbw_q, cw_t, sw_t, qs,
                                          h, t0, tcw, 1.0, epsq_sb)
                                # k tail + one v unit deferred behind the next
                                # head's matmul burst
                                deferred.append(
                                    lambda qsq=qsq_k, qbw=qbw_k, hh=h, tt0=t0,
                                           tw=tcw, c=cw_t, s=sw_t:
                                    norm_tail(None, qsq, qbw, c, s, ks, hh,
                                              tt0, tw, 1.0 / HD, epsk_sb)
                                )
                                if vtiles:
                                    tt, ttw, ti = vtiles.pop(0)
                                    deferred.append(
                                        lambda xt=xt, tt=tt, ttw=ttw, ti=ti:
                                        v_unit(xt, tt, ttw, ti)
                                    )
                                drain()
                            # leftover v tiles of this chunk
                            while vtiles:
                                tt, ttw, ti = vtiles.pop(0)
                                v_unit(xt, tt, ttw, ti)
                            t0 += tcw
                        drain()

                    if phases < 2:
                        continue
                    # ============ PHASE 2+3: attention + out proj ============
                    with tc.tile_pool(name=f"p2wo{it}", bufs=1) as wop, \
                         tc.tile_pool(name=f"p2ct{it}", bufs=2) as ctp, \
                         tc.tile_pool(name=f"p2pt{it}", bufs=3) as ptp, \
                         tc.tile_pool(name=f"p2v{it}", bufs=2) as vecp, \
                         tc.tile_pool(name=f"p2o{it}", bufs=4) as outp, \
                         tc.tile_pool(name=f"p2st{it}", bufs=3, space="PSUM") as psS, \
                         tc.tile_pool(name=f"p2cd{it}", bufs=1, space="PSUM") as psC:
                        wo_sb = wop.tile([P, HPC, DIM], BF16)
                        nc.sync.dma_start(wo_sb[:], wo.rearrange("(h p) o -> p h o", p=P))

                        out_units = []

                        def out_unit(ct3, uq0, tt, ttw, oc):
                            po = psS.tile([P, 1024], F32, tag="st")
                            for h in range(HPC):
                                nc.tensor.matmul(
                                    po[:ttw, :512],
                                    ct3[:, h, tt : tt + ttw],
                                    wo_sb[:, h, oc * 512 : (oc + 1) * 512],
                                    start=(h == 0), stop=(h == HPC - 1),
                                )
                            ot = outp.tile([P, 512], F32, tag="ot")
                            nc.vector.tensor_scalar_mul(ot[:ttw, :], po[:ttw, :512], 1.0)
                            nc.sync.dma_start(
                                out[uq0 + tt : uq0 + tt + ttw,
                                    oc * 512 : (oc + 1) * 512],
                                ot[:ttw, :],
                            )

                        for q0, qcw, klen in qcs:
                            kts = _ktiles(klen)
                            # finish any out-proj work from two chunks back
                            # before its ct3 slot gets rewritten
                            while out_units:
                                out_units.pop(0)()
                            ct3 = ctp.tile([P, HPC, 512], BF16, tag="ct3")
                            for h in range(HPC):
                                ct_ps = psC.tile([P, 512], F32, tag="ct")
                                dn_ps = psC.tile([P, 512], F32, tag="dn")
                                for i0 in range(0, len(kts), 2):
                                    pair = kts[i0 : i0 + 2]
                                    st = psS.tile([P, 1024], F32, tag="st")
                                    for j, (k0, ksz) in enumerate(pair):
                                        nc.tensor.matmul(
                                            st[:ksz, j * 512 : j * 512 + qcw],
                                            ks[:, h, k0 : k0 + ksz],
                                            qs[:, h, q0 : q0 + qcw],
                                            start=True, stop=True,
                                        )
                                    pt = ptp.tile([P, 1024], BF16, tag="pt")
                                    if len(pair) == 2 and pair[0][1] == pair[1][1] and qcw == 512:
                                        nc.scalar.activation(
                                            pt[: pair[0][1], :], st[: pair[0][1], :],
                                            mybir.ActivationFunctionType.Exp,
                                        )
                                    else:
                                        for j, (k0, ksz) in enumerate(pair):
                                            nc.scalar.activation(
                                                pt[:ksz, j * 512 : j * 512 + qcw],
                                                st[:ksz, j * 512 : j * 512 + qcw],
                                                mybir.ActivationFunctionType.Exp,
                                            )
                                    for j, (k0, ksz) in enumerate(pair):
                                        i = i0 + j
                                        nc.tensor.matmul(
                                            ct_ps[:, :qcw],
                                            vn[:ksz, i, h * HD : (h + 1) * HD],
                                            pt[:ksz, j * 512 : j * 512 + qcw],
                                            start=(i == 0),
                                            stop=(i == len(kts) - 1),
                                        )
                                        nc.tensor.matmul(
                                            dn_ps[:, :qcw],
                                            ones_bf[:ksz, :],
                                            pt[:ksz, j * 512 : j * 512 + qcw],
                                            start=(i == 0),
                                            stop=(i == len(kts) - 1),
                                        )
                                    # fill exp-wait gaps with prior chunk's
                                    # out-projection matmuls
                                    if out_units:
                                        out_units.pop(0)()
                                rcp2 = vecp.tile([P, 512], F32, tag="rcp2")
                                nc.vector.reciprocal(rcp2[:, :qcw], dn_ps[:, :qcw])
                                nc.vector.tensor_tensor(
                                    ct3[:, h, :qcw], ct_ps[:, :qcw], rcp2[:, :qcw],
                                    mybir.AluOpType.mult,
                                )
                            if phases < 3:
                                continue
                            for tt in range(0, qcw, P):
                                ttw = min(P, qcw - tt)
                                for oc in range(DIM // 512):
                                    out_units.append(
                                        lambda ct3=ct3, uq0=q0, tt=tt, ttw=ttw, oc=oc:
                                        out_unit(ct3, uq0, tt, ttw, oc)
                                    )
                        while out_units:
                            out_units.pop(0)()

    nc.compile()
    return nc


def _rope_tables():
    def axis_freqs(d, n):
        inv = 1.0 / (10000.0 ** (np.arange(0, d, 2, dtype=np.float32) / d))
        return np.arange(n, dtype=np.float32)[:, None] * inv[None, :]

    ft = np.broadcast_to(axis_freqs(D_T, T)[:, None, None, :], (T, H, W, D_T // 2))
    fh = np.broadcast_to(axis_freqs(D_H, H)[None, :, None, :], (T, H, W, D_H // 2))
    fw = np.broadcast_to(axis_freqs(D_W, W)[None, None, :, :], (T, H, W, D_W // 2))
    f = np.concatenate([ft, fh, fw], axis=-1).reshape(N, HD // 2)
    cos = np.cos(f).astype(np.float32)
    sin = np.sin(f).astype(np.float32)
    cwt = np.repeat(cos.T, 2, axis=0)
    swt = np.empty((HD, N), np.float32)
    swt[0::2] = -sin.T
    swt[1::2] = sin.T
    return cwt, swt


_NC_CACHE = {}


def _bf16(a):
    import ml_dtypes

    return np.asarray(a, dtype=ml_dtypes.bfloat16)


def prepare_in_maps(inputs):
    x = np.asarray(inputs["x"], dtype=np.float32)
    Wq = np.asarray(inputs["Wq"], dtype=np.float32)
    bq = np.asarray(inputs["bq"], dtype=np.float32)
    Wk = np.asarray(inputs["Wk"], dtype=np.float32)
    bk = np.asarray(inputs["bk"], dtype=np.float32)
    Wv = np.asarray(inputs["Wv"], dtype=np.float32)
    bv = np.asarray(inputs["bv"], dtype=np.float32)
    Wo = np.asarray(inputs["Wo"], dtype=np.float32)
    qnw = np.asarray(inputs["q_norm_w"], dtype=np.float32)
    knw = np.asarray(inputs["k_norm_w"], dtype=np.float32)

    xT = _bf16(np.ascontiguousarray(x.reshape(N, DIM).T))
    cwt, swt = _rope_tables()
    ones_bf = _bf16(np.ones((P, P), np.float32))
    ones_f = np.ones((P, P), np.float32)

    def headcols(vec, c):
        return np.ascontiguousarray(vec[c * HW_ : (c + 1) * HW_].reshape(HPC, HD).T)

    in_maps = []
    for c in range(NCORES):
        sl = slice(c * HW_, (c + 1) * HW_)
        in_maps.append(
            {
                "xT": xT,
                "wq": _bf16(Wq[:, sl]),
                "wk": _bf16(Wk[:, sl]),
                "wv": _bf16(Wv[:, sl]),
                "wo": _bf16(Wo[sl, :]),
                "cw": cwt,
                "sw": swt,
                "nwq": qnw.reshape(P, 1),
                "nwk": knw.reshape(P, 1),
                "bqc": headcols(bq, c),
                "bkc": headcols(bk, c),
                "wbq": headcols(bq, c) * qnw.reshape(P, 1),
                "wbk": headcols(bk, c) * knw.reshape(P, 1),
                "bvr": _bf16(bv[sl].reshape(1, HW_)),
                "ones_bf": ones_bf,
                "ones_f": ones_f,
            }
        )
    return in_maps


def kernel(**inputs):
    bo = np.asarray(inputs["bo"], dtype=np.float32)
    nct = int(inputs["num_cond_latents"]) * (N // T)

    if nct not in _NC_CACHE:
        _NC_CACHE[nct] = build_nc(nct)
    nc = _NC_CACHE[nct]

    in_maps = prepare_in_maps(inputs)
    res = run_bass_kernel_spmd(nc, in_maps, core_ids=list(range(NCORES)))
    acc = res.results[0]["out"].astype(np.float32)
    for c in range(1, NCORES):
        acc = acc + res.results[c]["out"]
    return (acc + bo).astype(np.float32).reshape(B, N, DIM)


if __name__ == "__main__":
    build_nc(800)
    print("build ok")


# revision 3
# speedup vs baseline: 85478.6164x; 1.0537x over previous
"""Bass/Tile TRN2 kernel for nn_LongCatSelfAttention (8-core head-parallel, fused
+ software-pipelined emission).

Same algorithm as kernel_v2 (bf16 matmuls, SBUF-resident q/k/v, fused
attention + out-projection), plus deferred-emission scheduling to keep the
TensorEngine busy through ScalarE (exp / rmsnorm) latencies:
  - phase 1: the k-projection rms matmul + tail and the v-projection units
    are deferred one head so the PE never waits on ACT.
  - phase 2/3: out-projection units of query-chunk j-1 are interleaved into
    the exp-wait gaps of chunk j's attention; PSUM drains happen on VectorE
    (ACT is the attention bottleneck).
"""

import numpy as np

import concourse.bacc as bacc
import concourse.mybir as mybir
import concourse.tile as tile
from concourse.bass_utils import run_bass_kernel_spmd

F32 = mybir.dt.float32
F32R = mybir.dt.float32r
BF16 = mybir.dt.bfloat16

B = 1
T, H, W = 8, 20, 20
N = T * H * W  # 3200
DIM = 3072
NH = 24
HD = 128
D_T, D_H, D_W = 32, 48, 48
EPS = 1e-6
NCORES = 8
HPC = NH // NCORES
HW_ = HPC * HD  # 384
KSUB = DIM // 128  # 24
P = 128

_PAIR_SWAP_MASK = [j ^ 1 for j in range(32)]

P1_CHUNKS = [384] * 8 + [128]
P1_MAX = max(P1_CHUNKS)
assert sum(P1_CHUNKS) == N


def _ktiles(total):
    return [(k0, min(P, total - k0)) for k0 in range(0, total, P)]


def _qchunks(nct):
    out = []
    for seg0, seglen, klen in ((0, nct, nct), (nct, N - nct, N)):
        q0 = seg0
        while q0 < seg0 + seglen:
            qlen = min(512, seg0 + seglen - q0)
            out.append((q0, qlen, klen))
            q0 += qlen
    return out


def build_nc(nct, iters=1, phases=3):
    nc = bacc.Bacc()

    xT = nc.dram_tensor("xT", [DIM, N], BF16, kind="ExternalInput")
    wq = nc.dram_tensor("wq", [DIM, HW_], BF16, kind="ExternalInput")
    wk = nc.dram_tensor("wk", [DIM, HW_], BF16, kind="ExternalInput")
    wv = nc.dram_tensor("wv", [DIM, HW_], BF16, kind="ExternalInput")
    wo = nc.dram_tensor("wo", [HW_, DIM], BF16, kind="ExternalInput")
    cw = nc.dram_tensor("cw", [P, N], BF16, kind="ExternalInput")
    sw = nc.dram_tensor("sw", [P, N], BF16, kind="ExternalInput")
    nwq = nc.dram_tensor("nwq", [P, 1], F32, kind="ExternalInput")
    nwk = nc.dram_tensor("nwk", [P, 1], F32, kind="ExternalInput")
    bqc = nc.dram_tensor("bqc", [P, HPC], F32, kind="ExternalInput")
    bkc = nc.dram_tensor("bkc", [P, HPC], F32, kind="ExternalInput")
    wbq = nc.dram_tensor("wbq", [P, HPC], F32, kind="ExternalInput")
    wbk = nc.dram_tensor("wbk", [P, HPC], F32, kind="ExternalInput")
    bvr = nc.dram_tensor("bvr", [1, HW_], BF16, kind="ExternalInput")
    ones_bf_in = nc.dram_tensor("ones_bf", [P, P], BF16, kind="ExternalInput")
    ones_f_in = nc.dram_tensor("ones_f", [P, P], F32R, kind="ExternalInput")
    out = nc.dram_tensor("out", [N, DIM], F32, kind="ExternalOutput")

    qcs = _qchunks(nct)
    n_vt = N // P

    with tile.TileContext(nc) as tc:
        with tc.tile_pool(name="const", bufs=1) as cpool:
            ones_bf = cpool.tile([P, P], BF16)
            ones_f = cpool.tile([P, P], F32R)
            nwq_sb = cpool.tile([P, 1], F32)
            nwk_sb = cpool.tile([P, 1], F32)
            bqc_sb = cpool.tile([P, HPC], F32)
            bkc_sb = cpool.tile([P, HPC], F32)
            wbq_sb = cpool.tile([P, HPC], F32)
            wbk_sb = cpool.tile([P, HPC], F32)
            bvr_sb = cpool.tile([1, HW_], BF16)
            nc.sync.dma_start(ones_bf[:], ones_bf_in[:])
            nc.sync.dma_start(ones_f[:], ones_f_in[:])
            nc.sync.dma_start(nwq_sb[:], nwq[:])
            nc.sync.dma_start(nwk_sb[:], nwk[:])
            nc.sync.dma_start(bqc_sb[:], bqc[:])
            nc.sync.dma_start(bkc_sb[:], bkc[:])
            nc.sync.dma_start(wbq_sb[:], wbq[:])
            nc.sync.dma_start(wbk_sb[:], wbk[:])
            nc.sync.dma_start(bvr_sb[:], bvr[:])
            epsq_sb = cpool.tile([P, 1], F32)
            epsk_sb = cpool.tile([P, 1], F32)
            nc.vector.memset(epsq_sb[:], float(HD) * EPS)
            nc.vector.memset(epsk_sb[:], EPS)

            for it in range(iters):
                with tc.tile_pool(name=f"qkv{it}", bufs=1) as qkvpool:
                    qs = qkvpool.tile([P, HPC, N], BF16, tag="qs")
                    ks = qkvpool.tile([P, HPC, N], BF16, tag="ks")
                    vn = qkvpool.tile([P, n_vt, HW_], BF16, tag="vn")

                    # ============ PHASE 1: QKV + norm + rope ============
                    with tc.tile_pool(name=f"p1w{it}", bufs=1) as wpool, \
                         tc.tile_pool(name=f"p1x{it}", bufs=2) as xpool, \
                         tc.tile_pool(name=f"p1t{it}", bufs=2) as tpool, \
                         tc.tile_pool(name=f"p1k{it}", bufs=3) as kpool, \
                         tc.tile_pool(name=f"p1ps{it}", bufs=2, space="PSUM") as pspool, \
                         tc.tile_pool(name=f"p1ps2{it}", bufs=2, space="PSUM") as pspool2, \
                         tc.tile_pool(name=f"p1psv{it}", bufs=2, space="PSUM") as psvpool:
                        wq_sb = wpool.tile([P, KSUB, HW_], BF16, tag="wq")
                        wk_sb = wpool.tile([P, KSUB, HW_], BF16, tag="wk")
                        wv_sb = wpool.tile([P, KSUB, HW_], BF16, tag="wv")
                        wqr = wq.rearrange("(ko p) m -> p ko m", p=P)
                        wkr = wk.rearrange("(ko p) m -> p ko m", p=P)
                        wvr = wv.rearrange("(ko p) m -> p ko m", p=P)
                        for kg in range(0, KSUB, 4):
                            nc.sync.dma_start(wq_sb[:, kg : kg + 4, :], wqr[:, kg : kg + 4, :])
                            nc.sync.dma_start(wk_sb[:, kg : kg + 4, :], wkr[:, kg : kg + 4, :])
                            nc.sync.dma_start(wv_sb[:, kg : kg + 4, :], wvr[:, kg : kg + 4, :])

                        deferred = []

                        def drain():
                            while deferred:
                                deferred.pop(0)()

                        def norm_tail(ps, qsq, qbw, cw_t, sw_t, dst, h, t0, tcw,
                                      sq_scale, sq_bias):
                            # rms matmul + sqrt/recip + rope, emitted as a unit
                            ssq = pspool2.tile([P, P1_MAX], F32, tag="ssq")
                            nc.tensor.matmul(
                                ssq[:, :tcw], ones_f[:], qsq[:, :tcw],
                                start=True, stop=True,
                            )
                            rmst = kpool.tile([P, P1_MAX], F32, tag="rms")
                            nc.scalar.activation(
                                rmst[:, :tcw], ssq[:, :tcw],
                                mybir.ActivationFunctionType.Sqrt,
                                bias=sq_bias[:], scale=sq_scale,
                            )
                            rcp = kpool.tile([P, P1_MAX], F32, tag="rcp")
                            nc.vector.reciprocal(rcp[:, :tcw], rmst[:, :tcw])
                            qsw = kpool.tile([P, P1_MAX], BF16, tag="qsw")
                            nc.vector.stream_shuffle(
                                qsw[:, :tcw], qbw[:, :tcw], _PAIR_SWAP_MASK
                            )
                            m1 = kpool.tile([P, P1_MAX], BF16, tag="m1")
                            nc.vector.tensor_tensor(
                                m1[:, :tcw], qbw[:, :tcw], cw_t[:, :tcw],
                                mybir.AluOpType.mult,
                            )
                            m2 = kpool.tile([P, P1_MAX], BF16, tag="m2")
                            nc.vector.tensor_tensor(
                                m2[:, :tcw], qsw[:, :tcw], sw_t[:, :tcw],
                                mybir.AluOpType.mult,
                            )
                            nc.vector.tensor_tensor(
                                m1[:, :tcw], m1[:, :tcw], m2[:, :tcw],
                                mybir.AluOpType.add,
                            )
                            nc.vector.tensor_tensor(
                                dst[:, h, t0 : t0 + tcw], m1[:, :tcw],
                                rcp[:, :tcw], mybir.AluOpType.mult,
                            )

                        def v_unit(xt, tt, ttw, ti):
                            psv = psvpool.tile([P, HW_], F32, tag="v")
                            for k in range(KSUB):
                                nc.tensor.matmul(
                                    psv[:ttw, :], xt[:, k, tt : tt + ttw],
                                    wv_sb[:, k, :], start=(k == 0), stop=False,
                                )
                            nc.tensor.matmul(
                                psv[:ttw, :], ones_bf[:1, :ttw], bvr_sb[:],
                                start=False, stop=True,
                            )
                            nc.scalar.copy(vn[:ttw, ti, :], psv[:ttw, :])

                        t0 = 0
                        for tcw in P1_CHUNKS:
                            xt = xpool.tile([P, KSUB, P1_MAX], BF16, tag="x")
                            xr = xT.rearrange("(ko p) n -> p ko n", p=P)
                            for kg in range(0, KSUB, 6):
                                nc.sync.dma_start(
                                    xt[:, kg : kg + 6, :tcw],
                                    xr[:, kg : kg + 6, t0 : t0 + tcw],
                                )
                            cw_t = tpool.tile([P, P1_MAX], BF16, tag="cw")
                            sw_t = tpool.tile([P, P1_MAX], BF16, tag="sw")
                            nc.sync.dma_start(cw_t[:, :tcw], cw[:, t0 : t0 + tcw])
                            nc.sync.dma_start(sw_t[:, :tcw], sw[:, t0 : t0 + tcw])

                            vtiles = [(tt, min(P, tcw - tt), (t0 + tt) // P)
                                      for tt in range(0, tcw, P)]
                            for h in range(HPC):
                                # qk projection matmuls back to back
                                ps_q = pspool.tile([P, P1_MAX], F32, tag="psq")
                                for k in range(KSUB):
                                    nc.tensor.matmul(
                                        ps_q[:, :tcw],
                                        wq_sb[:, k, h * HD : (h + 1) * HD],
                                        xt[:, k, :tcw],
                                        start=(k == 0), stop=(k == KSUB - 1),
                                    )
                                ps_k = pspool.tile([P, P1_MAX], F32, tag="psk")
                                for k in range(KSUB):
                                    nc.tensor.matmul(
                                        ps_k[:, :tcw],
                                        wk_sb[:, k, h * HD : (h + 1) * HD],
                                        xt[:, k, :tcw],
                                        start=(k == 0), stop=(k == KSUB - 1),
                                    )
                                # ACT ops for q then k (overlap the matmuls above)
                                qbw_q = kpool.tile([P, P1_MAX], BF16, tag="qbw")
                                nc.scalar.activation(
                                    qbw_q[:, :tcw], ps_q[:, :tcw],
                                    mybir.ActivationFunctionType.Identity,
                                    bias=wbq_sb[:, h : h + 1], scale=nwq_sb[:],
                                )
                                qsq_q = kpool.tile([P, P1_MAX], F32R, tag="qsq")
                                nc.scalar.activation(
                                    qsq_q[:, :tcw], ps_q[:, :tcw],
                                    mybir.ActivationFunctionType.Square,
                                    bias=bqc_sb[:, h : h + 1],
                                )
                                qbw_k = kpool.tile([P, P1_MAX], BF16, tag="qbw")
                                nc.scalar.activation(
                                    qbw_k[:, :tcw], ps_k[:, :tcw],
                                    mybir.ActivationFunctionType.Identity,
                                    bias=wbk_sb[:, h : h + 1], scale=nwk_sb[:],
                                )
                                qsq_k = kpool.tile([P, P1_MAX], F32R, tag="qsq")
                                nc.scalar.activation(
                                    qsq_k[:, :tcw], ps_k[:, :tcw],
                                    mybir.ActivationFunctionType.Square,
                                    bias=bkc_sb[:, h : h + 1],
                                )
                                # q tail now (ACT had the k-matmul window)
                                norm_tail(ps_q, qsq_q, qbw_q, cw_t, sw_t, qs,
                                          h, t0, tcw, 1.0, epsq_sb)
                                # k tail + one v unit deferred behind the next
                                # head's matmul burst
                                deferred.append(
                                    lambda qsq=qsq_k, qbw=qbw_k, hh=h, tt0=t0,
                                           tw=tcw, c=cw_t, s=sw_t:
                                    norm_tail(None, qsq, qbw, c, s, ks, hh,
                                              tt0, tw, 1.0 / HD, epsk_sb)
                                )
                                if vtiles:
                                    tt, ttw, ti = vtiles.pop(0)
                                    deferred.append(
                                        lambda xt=xt, tt=tt, ttw=ttw, ti=ti:
                                        v_unit(xt, tt, ttw, ti)
                                    )
                                drain()
                            # leftover v tiles of this chunk
                            while vtiles:
                                tt, ttw, ti = vtiles.pop(0)
                                v_unit(xt, tt, ttw, ti)
                            t0 += tcw
                        drain()

                    if phases < 2:
                        continue
                    # ============ PHASE 2+3: attention + out proj ============
                    with tc.tile_pool(name=f"p2wo{it}", bufs=1) as wop, \
                         tc.tile_pool(name=f"p2ct{it}", bufs=2) as ctp, \
                         tc.tile_pool(name=f"p2pt{it}", bufs=3) as ptp, \
                         tc.tile_pool(name=f"p2v{it}", bufs=2) as vecp, \
                         tc.tile_pool(name=f"p2o{it}", bufs=4) as outp, \
                         tc.tile_pool(name=f"p2st{it}", bufs=3, space="PSUM") as psS, \
                         tc.tile_pool(name=f"p2cd{it}", bufs=1, space="PSUM") as psC:
                        wo_sb = wop.tile([P, HPC, DIM], BF16)
                        nc.sync.dma_start(wo_sb[:], wo.rearrange("(h p) o -> p h o", p=P))

                        out_units = []

                        def out_unit(ct3, uq0, tt, ttw, oc):
                            po = psS.tile([P, 1024], F32, tag="st")
                            for h in range(HPC):
                                nc.tensor.matmul(
                                    po[:ttw, :512],
                                    ct3[:, h, tt : tt + ttw],
                                    wo_sb[:, h, oc * 512 : (oc + 1) * 512],
                                    start=(h == 0), stop=(h == HPC - 1),
                                )
                            ot = outp.tile([P, 512], F32, tag="ot")
                            nc.vector.tensor_scalar_mul(ot[:ttw, :], po[:ttw, :512], 1.0)
                            nc.sync.dma_start(
                                out[uq0 + tt : uq0 + tt + ttw,
                                    oc * 512 : (oc + 1) * 512],
                                ot[:ttw, :],
                            )

                        for q0, qcw, klen in qcs:
                            kts = _ktiles(klen)
                            # finish any out-proj work from two chunks back
                            # before its ct3 slot gets rewritten
                            while out_units:
                                out_units.pop(0)()
                            ct3 = ctp.tile([P, HPC, 512], BF16, tag="ct3")
                            for h in range(HPC):
                                ct_ps = psC.tile([P, 512], F32, tag="ct")
                                dn_ps = psC.tile([P, 512], F32, tag="dn")
                                for i0 in range(0, len(kts), 2):
                                    pair = kts[i0 : i0 + 2]
                                    st = psS.tile([P, 1024], F32, tag="st")
                                    for j, (k0, ksz) in enumerate(pair):
                                        nc.tensor.matmul(
                                            st[:ksz, j * 512 : j * 512 + qcw],
                                            ks[:, h, k0 : k0 + ksz],
                                            qs[:, h, q0 : q0 + qcw],
                                            start=True, stop=True,
                                        )
                                    pt = ptp.tile([P, 1024], BF16, tag="pt")
                                    if len(pair) == 2 and pair[0][1] == pair[1][1] and qcw == 512:
                                        nc.scalar.activation(
                                            pt[: pair[0][1], :], st[: pair[0][1], :],
                                            mybir.ActivationFunctionType.Exp,
                                        )
                                    else:
                                        for j, (k0, ksz) in enumerate(pair):
                                            nc.scalar.activation(
                                                pt[:ksz, j * 512 : j * 512 + qcw],
                                                st[:ksz, j * 512 : j * 512 + qcw],
                                                mybir.ActivationFunctionType.Exp,
                                            )
                                    for j, (k0, ksz) in enumerate(pair):
                                        i = i0 + j
                                        nc.tensor.matmul(
                                            ct_ps[:, :qcw],
                                            vn[:ksz, i, h * HD : (h + 1) * HD],
                                            pt[:ksz, j * 512 : j * 512 + qcw],
                                            start=(i == 0),
                                            stop=(i == len(kts) - 1),
                                        )
                                        nc.tensor.matmul(
                                            dn_ps[:, :qcw],
                                            ones_bf[:ksz, :],
                                            pt[:ksz, j * 512 : j * 512 + qcw],
                                            start=(i == 0),
                                            stop=(i == len(kts) - 1),
                                        )
                                    # fill exp-wait gaps with prior chunk's
                                    # out-projection matmuls
                                    if out_units:
                                        out_units.pop(0)()
                                rcp2 = vecp.tile([P, 512], F32, tag="rcp2")
                                nc.vector.reciprocal(rcp2[:, :qcw], dn_ps[:, :qcw])
                                nc.vector.tensor_tensor(
                                    ct3[:, h, :qcw], ct_ps[:, :qcw], rcp2[:, :qcw],
                                    mybir.AluOpType.mult,
                                )
                            if phases < 3:
                                continue
                            for tt in range(0, qcw, P):
                                ttw = min(P, qcw - tt)
                                for oc in range(DIM // 512):
                                    out_units.append(
                                        lambda ct3=ct3, uq0=q0, tt=tt, ttw=ttw, oc=oc:
                                        out_unit(ct3, uq0, tt, ttw, oc)
                                    )
                        while out_units:
                            out_units.pop(0)()

    nc.compile()
    return nc


def _rope_tables():
    def axis_freqs(d, n):
        inv = 1.0 / (10000.0 ** (np.arange(0, d, 2, dtype=np.float32) / d))
        return np.arange(n, dtype=np.float32)[:, None] * inv[None, :]

    ft = np.broadcast_to(axis_freqs(D_T, T)[:, None, None, :], (T, H, W, D_T // 2))
    fh = np.broadcast_to(axis_freqs(D_H, H)[None, :, None, :], (T, H, W, D_H // 2))
    fw = np.broadcast_to(axis_freqs(D_W, W)[None, None, :, :], (T, H, W, D_W // 2))
    f = np.concatenate([ft, fh, fw], axis=-1).reshape(N, HD // 2)
    cos = np.cos(f).astype(np.float32)
    sin = np.sin(f).astype(np.float32)
    cwt = np.repeat(cos.T, 2, axis=0)
    swt = np.empty((HD, N), np.float32)
    swt[0::2] = -sin.T
    swt[1::2] = sin.T
    return cwt, swt


_NC_CACHE = {}


def _bf16(a):
    import ml_dtypes

    return np.asarray(a, dtype=ml_dtypes.bfloat16)


def prepare_in_maps(inputs):
    x = np.asarray(inputs["x"], dtype=np.float32)
    Wq = np.asarray(inputs["Wq"], dtype=np.float32)
    bq = np.asarray(inputs["bq"], dtype=np.float32)
    Wk = np.asarray(inputs["Wk"], dtype=np.float32)
    bk = np.asarray(inputs["bk"], dtype=np.float32)
    Wv = np.asarray(inputs["Wv"], dtype=np.float32)
    bv = np.asarray(inputs["bv"], dtype=np.float32)
    Wo = np.asarray(inputs["Wo"], dtype=np.float32)
    qnw = np.asarray(inputs["q_norm_w"], dtype=np.float32)
    knw = np.asarray(inputs["k_norm_w"], dtype=np.float32)

    xT = _bf16(np.ascontiguousarray(x.reshape(N, DIM).T))
    cwt, swt = _rope_tables()
    ones_bf = _bf16(np.ones((P, P), np.float32))
    ones_f = np.ones((P, P), np.float32)

    def headcols(vec, c):
        return np.ascontiguousarray(vec[c * HW_ : (c + 1) * HW_].reshape(HPC, HD).T)

    in_maps = []
    for c in range(NCORES):
        sl = slice(c * HW_, (c + 1) * HW_)
        in_maps.append(
            {
                "xT": xT,
                "wq": _bf16(Wq[:, sl]),
                "wk": _bf16(Wk[:, sl]),
                "wv": _bf16(Wv[:, sl]),
                "wo": _bf16(Wo[sl, :]),
                "cw": _bf16(cwt),
                "sw": _bf16(swt),
                "nwq": qnw.reshape(P, 1),
                "nwk": knw.reshape(P, 1),
                "bqc": headcols(bq, c),
                "bkc": headcols(bk, c),
                "wbq": headcols(bq, c) * qnw.reshape(P, 1),
                "wbk": headcols(bk, c) * knw.reshape(P, 1),
                "bvr": _bf16(bv[sl].reshape(1, HW_)),
                "ones_bf": ones_bf,
                "ones_f": ones_f,
            }
        )
    return in_maps


def kernel(**inputs):
    bo = np.asarray(inputs["bo"], dtype=np.float32)
    nct = int(inputs["num_cond_latents"]) * (N // T)

    if nct not in _NC_CACHE:
        _NC_CACHE[nct] = build_nc(nct)
    nc = _NC_CACHE[nct]

    in_maps = prepare_in_maps(inputs)
    res = run_bass_kernel_spmd(nc, in_maps, core_ids=list(range(NCORES)))
    acc = res.results[0]["out"].astype(np.float32)
    for c in range(1, NCORES):
        acc = acc + res.results[c]["out"]
    return (acc + bo).astype(np.float32).reshape(B, N, DIM)


if __name__ == "__main__":
    build_nc(800)
    print("build ok")
